# revision 1
# baseline (speedup 1.0000x reference)
"""Trainium2 Bass kernel for nn_LlamaAttention_45749991637119.

Mathematical structure of the reference: K/V are a single shared head that
is broadcast across all 64 query heads, and attention is computed per token
position (no cross-token mixing).  scores[b,t,h,g] = q[b,t,h]·k[b,t] is
independent of g, so the softmax over g is exactly uniform (1/64) and
attn[b,t,h,:] == v[b,t,:] for every head h.  Therefore

    out = (hidden @ Wv.T) @ Wo_sum.T,   Wo_sum[i,d] = sum_h Wo[i, 64h+d]

and Wq/Wk/cos/sin never influence the output (verified to 5e-7 rel err
against the reference).

Device work per core (1024 tokens):
  stage A:  vT[64, tok] = Wv @ hidden^T      (K=4096 contraction, PSUM accum)
  stage B:  out[tok, 4096] = vT.T @ WoSumT   (K=64 contraction)

Sharding: data-parallel over tokens (B*T = 8192 -> 1024 per core).  All
inputs are packed on the host into ONE [128, 38912] bf16 tensor (Wv^T
chunks | Wo_sum^T | hidden^T) so the kernel needs a single input DMA and a
single output DMA — this walrus build allows at most 1 sync-wait per DMA
and a very small wait list on the kernel-tail drain, so the semaphore
budget is the binding constraint.
"""

import numpy as np

import concourse.bass as bass
import concourse.mybir as mybir
from concourse.tile import TileContext
from concourse.bass_utils import run_bass_kernel_spmd

N_CORES = 8
B, T, HID = 4, 2048, 4096
D = 64                      # v dim (head_dim)
TOKS = (B * T) // N_CORES   # 1024 tokens per core
P = 128                     # partitions
TG = 512                    # token group = stage-A matmul free dim
CD = 512                    # stage-B out-column tile
KC = HID // P               # 32 k-chunks
NROW = TOKS // P            # 8 row-blocks of 128 tokens

# packed input column offsets (bf16 elements per partition)
WV_COLS = KC * D            # 2048
WOS_COLS = HID              # 4096
HT_COLS = KC * TOKS         # 32768
PACK_COLS = WV_COLS + WOS_COLS + HT_COLS

COMPUTE_DTYPE = "bf16"
_CACHE = {}
LAST_RESULT = None


def _build():
    dt_in = mybir.dt.bfloat16
    f32 = mybir.dt.float32

    nc = bass.Bass()
    pack = nc.dram_tensor("pack", [P, PACK_COLS], dt_in, kind="ExternalInput")
    out = nc.dram_tensor("out", [TOKS, HID], dt_in, kind="ExternalOutput")

    NB = 4  # stage-B psum ring
    with (
        nc.sbuf_tensor([P, PACK_COLS], dt_in) as mega,
        nc.sbuf_tensor([P, NROW * HID], dt_in) as out_sb,
        nc.sbuf_tensor([D, TOKS], dt_in) as vT,
        nc.psum_tensor([D, TG]) as psv0,
        nc.psum_tensor([D, TG]) as psv1,
        nc.psum_tensor([P, NB * CD]) as psB,
        nc.semaphore() as s_load,
        nc.semaphore() as s_pe,
        nc.semaphore() as s_dve,
        nc.semaphore() as s_store,
        nc.Block() as block,
    ):
        psv = [psv0, psv1]

        def wv_chunk(c):
            return mega[:, c * D:(c + 1) * D]

        def woS(ct):
            return mega[:D, WV_COLS + ct * CD:WV_COLS + (ct + 1) * CD]

        def ht(c, tok0, ntok):
            base = WV_COLS + WOS_COLS + c * TOKS + tok0
            return mega[:, base:base + ntok]

        NG = TOKS // TG          # 2 groups
        BPG = (TG // P) * (HID // CD)   # 32 stage-B matmuls per group

        NPC = 4                  # ht load pieces
        CPL = KC // NPC          # k-chunks per load piece
        HT0 = WV_COLS + WOS_COLS

        @block.sync
        def _(sync):
            # piece 0: weights (wv + woS), pieces 1..NPC: ht k-ranges
            sync.dma_start(out=mega[:, :HT0], in_=pack[:, :HT0]).then_inc(s_load, 16)
            for i in range(NPC):
                lo = HT0 + i * CPL * TOKS
                hi = HT0 + (i + 1) * CPL * TOKS
                sync.dma_start(out=mega[:, lo:hi], in_=pack[:, lo:hi]).then_inc(
                    s_load, 16
                )

        @block.tensor
        def _(tensor):
            pe_tick = 0
            for g in range(NG):
                tok0 = g * TG
                for c in range(KC):
                    if g == 0 and c % CPL == 0:
                        # weights piece + ht pieces up to this k-range
                        tensor.wait_ge(s_load, 16 * (2 + c // CPL))
                    mm = tensor.matmul(
                        psv[g][:, :], wv_chunk(c), ht(c, tok0, TG),
                        start=(c == 0), stop=(c == KC - 1),
                    )
                    if c == KC - 1:
                        mm.then_inc(s_pe, 1)
                pe_tick += 1
                # vT(g) ready tick on s_dve: g*(1+BPG)+1
                vt_tick = g * (1 + BPG) + 1
                i = 0
                for tt in range(TG // P):
                    rb = g * (TG // P) + tt
                    for ct in range(HID // CD):
                        slot = i % NB
                        # WAR: copy of the matmul 4 back must be done
                        war = g * (1 + BPG) + 1 + (i - NB + 1)
                        tensor.wait_ge(s_dve, max(vt_tick, war))
                        tensor.matmul(
                            psB[:, slot * CD:(slot + 1) * CD],
                            vT[:, rb * P:(rb + 1) * P],
                            woS(ct),
                            start=True, stop=True,
                        ).then_inc(s_pe, 1)
                        pe_tick += 1
                        i += 1

        @block.vector
        def _(vector):
            dve_tick = 0
            for g in range(NG):
                # vT copy: wait stage-A accumulation done (pe tick g*(1+BPG)+1)
                vector.wait_ge(s_pe, g * (1 + BPG) + 1)
                vector.tensor_copy(
                    out=vT[:, g * TG:(g + 1) * TG], in_=psv[g][:, :]
                ).then_inc(s_dve, 1)
                dve_tick += 1
                i = 0
                for tt in range(TG // P):
                    rb = g * (TG // P) + tt
                    for ct in range(HID // CD):
                        slot = i % NB
                        vector.wait_ge(s_pe, g * (1 + BPG) + 2 + i)
                        vector.tensor_copy(
                            out=out_sb[:, rb * HID + ct * CD:rb * HID + (ct + 1) * CD],
                            in_=psB[:, slot * CD:(slot + 1) * CD],
                        ).then_inc(s_dve, 1)
                        dve_tick += 1
                        i += 1

        @block.gpsimd
        def _(gpsimd):
            # store 2 row-blocks as soon as their copies land
            for s in range(NROW // 2):
                rb_hi = 2 * s + 1
                tick = (rb_hi // 4) * (1 + BPG) + 1 + ((rb_hi % 4) + 1) * (HID // CD)
                gpsimd.wait_ge(s_dve, tick)
                gpsimd.dma_start(
                    out=out[s * 2 * P:(s + 1) * 2 * P, :].rearrange(
                        "(r p) c -> p r c", p=P
                    ),
                    in_=out_sb[:, s * 2 * HID:(s + 1) * 2 * HID].rearrange(
                        "p (r c) -> p r c", r=2
                    ),
                ).then_inc(s_store, 16)
            gpsimd.wait_ge(s_store, 16 * (NROW // 2))
    return nc


def kernel(hidden_states, cos, sin, Wq, Wk, Wv, Wo):
    global LAST_RESULT
    import ml_dtypes
    np_bf16 = ml_dtypes.bfloat16

    if "nc" not in _CACHE:
        _CACHE["nc"] = _build()
    nc = _CACHE["nc"]

    hidden_states = np.asarray(hidden_states, dtype=np.float32)
    Wv = np.asarray(Wv, dtype=np.float32)
    Wo = np.asarray(Wo, dtype=np.float32)

    flat = hidden_states.reshape(B * T, HID)
    # Wv^T chunks: pack[p, c*64+d] = Wv[d, c*128+p]
    wv_part = np.ascontiguousarray(
        Wv.reshape(D, KC, P).transpose(2, 1, 0).reshape(P, KC * D)
    ).astype(np_bf16)
    # Wo_sum^T on partitions 0..63: pack[d, j] = sum_h Wo[j, 64h+d]
    woS = Wo.reshape(HID, HID // D, D).sum(axis=1, dtype=np.float32).T  # [64, 4096]
    woS_part = np.zeros((P, WOS_COLS), dtype=np_bf16)
    woS_part[:D, :] = woS.astype(np_bf16)

    in_maps = []
    for j in range(N_CORES):
        blk = flat[j * TOKS:(j + 1) * TOKS, :]          # [1024, 4096]
        # ht part: pack[p, c*1024+t] = blk[t, c*128+p]
        ht_part = np.ascontiguousarray(
            blk.reshape(TOKS, KC, P).transpose(2, 1, 0).reshape(P, KC * TOKS)
        ).astype(np_bf16)
        packed = np.concatenate([wv_part, woS_part, ht_part], axis=1)
        in_maps.append({"pack": np.ascontiguousarray(packed)})

    LAST_RESULT = run_bass_kernel_spmd(nc, in_maps, core_ids=list(range(N_CORES)))
    outs = [np.asarray(LAST_RESULT.results[j]["out"]).astype(np.float32)
            for j in range(N_CORES)]
    return np.concatenate(outs, axis=0).reshape(B, T, HID)



# revision 11
# speedup vs baseline: 1.4583x; 1.4583x over previous
"""Trainium2 Bass kernel for nn_LlamaAttention_45749991637119.

Mathematical structure of the reference: K/V are a single shared head that
is broadcast across all 64 query heads, and attention is computed per token
position (no cross-token mixing).  scores[b,t,h,g] = q[b,t,h]·k[b,t] is
independent of g, so the softmax over g is exactly uniform (1/64) and
attn[b,t,h,:] == v[b,t,:] for every head h.  Therefore

    out = (hidden @ Wv.T) @ Wo_sum.T,   Wo_sum[i,d] = sum_h Wo[i, 64h+d]

and Wq/Wk/cos/sin never influence the output (verified to 5e-7 rel err
against the reference).

Device work per core (1024 tokens), fully pipelined in G=4 token groups of
256 so compute overlaps the input stream:

  per group g:  stage A:  psv[64,256] += Wv_chunk.T @ ht(g,c)   (32 k-chunks)
                vT cast   psv -> vT sbuf            (Pool engine)
                stage B:  psB[128,512] = vT_rb.T @ WoSum_ct     (16 tiles)
                casts     psB -> out_sb  split Vector/Scalar/Pool by column
                store     1MB out row-block DMA from Pool's queue as soon as
                          its casts land

Input is ONE packed [128, 38912] bf16 DRAM tensor (Wv^T chunks | Wo_sum^T |
hidden^T group-major) streamed in 21 small pieces in exact consumption
order from the Sync engine's queue, so the first matmul starts ~10us in and
stage A is never starved.  Output DMAs ride Pool's separate queue so they
overlap the remaining input stream.  Engine split keeps the tensor engine
continuously busy (p-state ramp to 2.4 GHz) while three engines share the
PSUM->SBUF cast load.

Sharding: data-parallel over tokens (B*T = 8192 -> 1024 per core).
"""

from contextlib import ExitStack

import numpy as np

import concourse.bass as bass
import concourse.mybir as mybir
from concourse.bass_utils import run_bass_kernel_spmd

N_CORES = 8
B, T, HID = 4, 2048, 4096
D = 64                      # v dim (head_dim)
TOKS = (B * T) // N_CORES   # 1024 tokens per core
P = 128                     # partitions
G = 4                       # token groups per core
TGK = TOKS // G             # 256 tokens per group
KC = HID // P               # 32 k-chunks
CPL = 8                     # k-chunks per ht DMA piece
NPP = KC // CPL             # 4 ht pieces per group
NROW = TOKS // P            # 8 output row-blocks of 128 tokens
CD = 512                    # stage-B out-column tile
NCT = HID // CD             # 8 column tiles per row-block
NB = 6                      # stage-B psum ring slots (6 banks)

# packed input column offsets (bf16 elements per partition)
WV_COLS = KC * D            # 2048
WOS_COLS = HID              # 4096
HT0 = WV_COLS + WOS_COLS    # 6144
HT_COLS = KC * TOKS         # 32768
PACK_COLS = HT0 + HT_COLS   # 38912

# cast-engine assignment per column tile: V=Vector, A=Scalar
# (GpSimd/Pool cannot access PSUM in this walrus build, so it only runs the
# output DMA queue)
ENG_OF_CT = ["V", "A", "V", "A", "V", "A", "V", "A"]

COMPUTE_DTYPE = "bf16"
_CACHE = {}
LAST_RESULT = None


def _piece_plan():
    """Input DMA pieces in issue order -> (lo, hi) column ranges."""
    pieces = []
    pieces.append(("wv", (0, WV_COLS)))
    for pc in range(NPP):  # g0 ht pieces interleaved with woS quarters
        lo = HT0 + pc * CPL * TGK
        pieces.append((("ht", 0, pc), (lo, lo + CPL * TGK)))
        wlo = WV_COLS + pc * (WOS_COLS // NPP)
        pieces.append((("wos", pc), (wlo, wlo + WOS_COLS // NPP)))
    for g in range(1, G):
        for pc in range(NPP):
            lo = HT0 + g * KC * TGK + pc * CPL * TGK
            pieces.append((("ht", g, pc), (lo, lo + CPL * TGK)))
    return pieces


def _build():
    dt_in = mybir.dt.bfloat16

    pieces = _piece_plan()
    piece_idx = {key: idx for idx, (key, _) in enumerate(pieces)}

    # stage-B tile bookkeeping: global tile i = g*16 + rb_local*8 + ct
    n_tiles = G * 2 * NCT
    eng_of_tile = [ENG_OF_CT[i % NCT] for i in range(n_tiles)]
    cum = {"V": [0] * n_tiles, "A": [0] * n_tiles}
    cnt = {"V": 0, "A": 0}
    for i in range(n_tiles):
        cnt[eng_of_tile[i]] += 1
        for e in ("V", "A"):
            cum[e][i] = cnt[e]

    def tick_a(g):          # s_pe value after stage A(g) accumulation done
        return g * 17 + 1

    def tick_b(g, k):       # s_pe value after stage-B matmul k of group g
        return g * 17 + 2 + k

    nc = bass.Bass()
    pack = nc.dram_tensor("pack", [P, PACK_COLS], dt_in, kind="ExternalInput")
    out = nc.dram_tensor("out", [TOKS, HID], dt_in, kind="ExternalOutput")

    with ExitStack() as ctx:
        mega = ctx.enter_context(nc.sbuf_tensor([P, PACK_COLS], dt_in))
        out_sb = ctx.enter_context(nc.sbuf_tensor([P, NROW * HID], dt_in))
        vT = ctx.enter_context(nc.sbuf_tensor([D, TOKS], dt_in))
        psv0 = ctx.enter_context(nc.psum_tensor([D, TGK]))
        psv1 = ctx.enter_context(nc.psum_tensor([D, TGK]))
        psB = ctx.enter_context(nc.psum_tensor([P, NB * CD]))
        # one semaphore per input piece: a shared counting semaphore races
        # (the 16 per-engine completion increments of piece k+1 are
        # indistinguishable from piece k's), so "s >= 16*(k+1)" does not
        # guarantee piece k fully landed under engine skew.
        s_piece = [
            ctx.enter_context(nc.semaphore(name=f"s_piece{i}"))
            for i in range(len(pieces))
        ]
        s_pe = ctx.enter_context(nc.semaphore())
        s_vt = ctx.enter_context(nc.semaphore())
        s_cd = ctx.enter_context(nc.semaphore())
        s_ca = ctx.enter_context(nc.semaphore())
        s_store = ctx.enter_context(nc.semaphore())
        block = ctx.enter_context(nc.Block())

        psv = [psv0, psv1]
        sem_of = {"V": s_cd, "A": s_ca}

        def wv_chunk(c):
            return mega[:, c * D:(c + 1) * D]

        def wos_ap(ct):
            return mega[:D, WV_COLS + ct * CD:WV_COLS + (ct + 1) * CD]

        def ht_ap(g, c):
            lo = HT0 + g * KC * TGK + c * TGK
            return mega[:, lo:lo + TGK]

        def tile_aps(g, k):
            rb_l, ct = divmod(k, NCT)
            i = g * 2 * NCT + k
            slot = i % NB
            rb = g * 2 + rb_l
            dst = out_sb[:, rb * HID + ct * CD:rb * HID + (ct + 1) * CD]
            src = psB[:, slot * CD:(slot + 1) * CD]
            return dst, src

        @block.sync
        def _(sync):
            for idx, (key, (lo, hi)) in enumerate(pieces):
                sync.dma_start(out=mega[:, lo:hi], in_=pack[:, lo:hi]).then_inc(
                    s_piece[idx], 16
                )

        @block.tensor
        def _(tensor):
            for g in range(G):
                # stage A: accumulate vT(g) over 32 k-chunks as pieces land
                if g >= 2:
                    tensor.wait_ge(s_vt, g - 1)  # WAR: psv[g%2] cast done
                for c in range(KC):
                    if c % CPL == 0:
                        if g == 0 and c == 0:
                            tensor.wait_ge(s_piece[piece_idx["wv"]], 16)
                        tensor.wait_ge(s_piece[piece_idx[("ht", g, c // CPL)]], 16)
                    mm = tensor.matmul(
                        psv[g % 2][:, :], wv_chunk(c), ht_ap(g, c),
                        start=(c == 0), stop=(c == KC - 1),
                    )
                    if c == KC - 1:
                        mm.then_inc(s_pe, 1)
                # stage B: 2 row-blocks x 8 column tiles
                tensor.wait_ge(s_vt, g + 1)  # vT(g) cast landed in SBUF
                for k in range(2 * NCT):
                    rb_l, ct = divmod(k, NCT)
                    i = g * 2 * NCT + k
                    if g == 0 and rb_l == 0 and ct % 2 == 0:
                        tensor.wait_ge(s_piece[piece_idx[("wos", ct // 2)]], 16)
                    if i >= NB:
                        j = i - NB  # WAR: cast of ring predecessor done
                        tensor.wait_ge(sem_of[eng_of_tile[j]],
                                       cum[eng_of_tile[j]][j])
                    slot = i % NB
                    rb = g * 2 + rb_l
                    tensor.matmul(
                        psB[:, slot * CD:(slot + 1) * CD],
                        vT[:, rb * P:(rb + 1) * P],
                        wos_ap(ct),
                        start=True, stop=True,
                    ).then_inc(s_pe, 1)

        @block.vector
        def _(vector):
            for g in range(G):
                for k in range(2 * NCT):
                    if ENG_OF_CT[k % NCT] != "V":
                        continue
                    vector.wait_ge(s_pe, tick_b(g, k))
                    dst, src = tile_aps(g, k)
                    vector.tensor_copy(out=dst, in_=src).then_inc(s_cd, 1)

        @block.scalar
        def _(scalar):
            for g in range(G):
                scalar.wait_ge(s_pe, tick_a(g))
                scalar.copy(
                    out=vT[:, g * TGK:(g + 1) * TGK], in_=psv[g % 2][:, :]
                ).then_inc(s_vt, 1)
                for k in range(2 * NCT):
                    if ENG_OF_CT[k % NCT] != "A":
                        continue
                    scalar.wait_ge(s_pe, tick_b(g, k))
                    dst, src = tile_aps(g, k)
                    scalar.copy(out=dst, in_=src).then_inc(s_ca, 1)

        @block.gpsimd
        def _(gpsimd):
            for rb in range(NROW):
                gpsimd.wait_ge(s_cd, 4 * (rb + 1))
                gpsimd.wait_ge(s_ca, 4 * (rb + 1))
                gpsimd.dma_start(
                    out=out[rb * P:(rb + 1) * P, :],
                    in_=out_sb[:, rb * HID:(rb + 1) * HID],
                ).then_inc(s_store, 16)
            gpsimd.wait_ge(s_store, 16 * NROW)
    return nc


def kernel(hidden_states, cos, sin, Wq, Wk, Wv, Wo):
    global LAST_RESULT
    import ml_dtypes
    np_bf16 = ml_dtypes.bfloat16

    if "nc" not in _CACHE:
        _CACHE["nc"] = _build()
    nc = _CACHE["nc"]

    hidden_states = np.asarray(hidden_states, dtype=np.float32)
    Wv = np.asarray(Wv, dtype=np.float32)
    Wo = np.asarray(Wo, dtype=np.float32)

    flat = hidden_states.reshape(B * T, HID)
    # Wv^T chunks: pack[p, c*64+d] = Wv[d, c*128+p]
    wv_part = np.ascontiguousarray(
        Wv.reshape(D, KC, P).transpose(2, 1, 0).reshape(P, KC * D)
    ).astype(np_bf16)
    # Wo_sum^T on partitions 0..63: pack[d, j] = sum_h Wo[j, 64h+d]
    woS = Wo.reshape(HID, HID // D, D).sum(axis=1, dtype=np.float32).T  # [64, 4096]
    woS_part = np.zeros((P, WOS_COLS), dtype=np_bf16)
    woS_part[:D, :] = woS.astype(np_bf16)

    in_maps = []
    for j in range(N_CORES):
        blk = flat[j * TOKS:(j + 1) * TOKS, :]          # [1024, 4096]
        # ht group-major: pack[p, g*KC*TGK + c*TGK + t'] = blk[g*TGK+t', c*128+p]
        ht_part = np.ascontiguousarray(
            blk.reshape(G, TGK, KC, P).transpose(3, 0, 2, 1).reshape(P, HT_COLS)
        ).astype(np_bf16)
        packed = np.concatenate([wv_part, woS_part, ht_part], axis=1)
        in_maps.append({"pack": np.ascontiguousarray(packed)})

    LAST_RESULT = run_bass_kernel_spmd(nc, in_maps, core_ids=list(range(N_CORES)))
    outs = [np.asarray(LAST_RESULT.results[j]["out"]).astype(np.float32)
            for j in range(N_CORES)]
    return np.concatenate(outs, axis=0).reshape(B, T, HID)


# revision 17
# speedup vs baseline: 1.5679x; 1.0751x over previous
"""Trainium2 Bass kernel for nn_LlamaAttention_45749991637119.

Mathematical structure of the reference: K/V are a single shared head that
is broadcast across all 64 query heads, and attention is computed per token
position (no cross-token mixing).  scores[b,t,h,g] = q[b,t,h]·k[b,t] is
independent of g, so the softmax over g is exactly uniform (1/64) and
attn[b,t,h,:] == v[b,t,:] for every head h.  Therefore

    out = (hidden @ Wv.T) @ Wo_sum.T,   Wo_sum[i,d] = sum_h Wo[i, 64h+d]

and Wq/Wk/cos/sin never influence the output (verified to 5e-7 rel err
against the reference).

Device work per core (1024 tokens), fully pipelined in G=4 token groups of
256 so compute overlaps the input stream.  PE order lags stage B one group
behind stage A (A0 A1 B0 A2 B1 A3 B2 B3) so the PSUM->SBUF vT cast of each
group never stalls the tensor engine:

  stage A(g):  psv[64,256] += Wv_chunk.T @ ht(g,c)   (32 k-chunks)
  vT cast(g):  psv -> vT sbuf                        (Scalar engine)
  stage B(g):  psB[128,512] = vT_rb.T @ WoSum_ct     (16 tiles, 6-bank ring)
  pair casts:  psB -> out_sb as [128,1024] 2-tile copies, alternating
               Vector / Scalar (GpSimd cannot touch PSUM in this build)
  store:       1MB out row-block DMAs from GpSimd's queue as casts land
               (separate queue so stores overlap the remaining input stream)

Input is ONE packed [128, 38912] bf16 DRAM tensor (Wv^T chunks | Wo_sum^T |
hidden^T group-major) streamed in 21 small pieces in exact consumption
order from the Sync engine's queue, so the first matmul starts ~10us in and
stage A is never starved.  Each piece gets its OWN semaphore: DMA completion
increments arrive per-DMA-engine (16 of them), so a shared counter cannot
distinguish piece k+1's increments from piece k's under engine skew (this
raced and corrupted results when pieces were small).  A dummy Scalar copy at
t=0 preloads the activation table (~1.5us) off the critical path.

Sharding: data-parallel over tokens (B*T = 8192 -> 1024 per core).
"""

from contextlib import ExitStack

import numpy as np

import concourse.bass as bass
import concourse.mybir as mybir
from concourse.bass_utils import run_bass_kernel_spmd

N_CORES = 8
B, T, HID = 4, 2048, 4096
D = 64                      # v dim (head_dim)
TOKS = (B * T) // N_CORES   # 1024 tokens per core
P = 128                     # partitions
G = 4                       # token groups per core
TGK = TOKS // G             # 256 tokens per group
KC = HID // P               # 32 k-chunks
CPL = 8                     # k-chunks per ht DMA piece
NPP = KC // CPL             # 4 ht pieces per group
NROW = TOKS // P            # 8 output row-blocks of 128 tokens
CD = 512                    # stage-B out-column tile
NCT = HID // CD             # 8 column tiles per row-block
NB = 6                      # stage-B psum ring slots (6 banks)

# packed input column offsets (bf16 elements per partition)
WV_COLS = KC * D            # 2048
WOS_COLS = HID              # 4096
HT0 = WV_COLS + WOS_COLS    # 6144
HT_COLS = KC * TOKS         # 32768
PACK_COLS = HT0 + HT_COLS   # 38912

# cast-engine assignment per column tile: V=Vector, A=Scalar
# (GpSimd/Pool cannot access PSUM in this walrus build, so it only runs the
# output DMA queue)
ENG_OF_CT = ["V", "A", "V", "A", "V", "A", "V", "A"]

COMPUTE_DTYPE = "bf16"
_CACHE = {}
LAST_RESULT = None


def _piece_plan():
    """Input DMA pieces in issue order -> (lo, hi) column ranges."""
    pieces = []
    pieces.append(("wv", (0, WV_COLS)))
    for pc in range(NPP):  # g0 ht pieces interleaved with woS quarters
        lo = HT0 + pc * CPL * TGK
        pieces.append((("ht", 0, pc), (lo, lo + CPL * TGK)))
        wlo = WV_COLS + pc * (WOS_COLS // NPP)
        pieces.append((("wos", pc), (wlo, wlo + WOS_COLS // NPP)))
    for g in range(1, G):
        for pc in range(NPP):
            lo = HT0 + g * KC * TGK + pc * CPL * TGK
            pieces.append((("ht", g, pc), (lo, lo + CPL * TGK)))
    return pieces


def _build():
    dt_in = mybir.dt.bfloat16

    pieces = _piece_plan()
    piece_idx = {key: idx for idx, (key, _) in enumerate(pieces)}

    # PE schedule: stage B lags one group behind stage A so the vT cast of
    # group g has a whole stage-A period to complete before B(g) needs it.
    ORDER = [("A", 0), ("A", 1), ("B", 0), ("A", 2), ("B", 1), ("A", 3),
             ("B", 2), ("B", 3)]
    tick_a, tick_b = {}, {}
    t = 0
    for kind, g in ORDER:
        if kind == "A":
            t += 1
            tick_a[g] = t
        else:
            for k in range(2 * NCT):
                t += 1
                tick_b[(g, k)] = t

    # stage-B casts run as PAIRS of adjacent tiles (one [128,1024] copy) to
    # amortize the fixed PSUM-access/decode cost.  global tile i = g*16 + k,
    # pair p = i//2, alternating Vector / Scalar.
    n_tiles = G * 2 * NCT
    n_pairs = n_tiles // 2
    eng_of_pair = ["V" if p % 2 == 0 else "A" for p in range(n_pairs)]
    pcum = {"V": [0] * n_pairs, "A": [0] * n_pairs}
    pcnt = {"V": 0, "A": 0}
    for p_ in range(n_pairs):
        pcnt[eng_of_pair[p_]] += 1
        for e in ("V", "A"):
            pcum[e][p_] = pcnt[e]

    nc = bass.Bass()
    pack = nc.dram_tensor("pack", [P, PACK_COLS], dt_in, kind="ExternalInput")
    out = nc.dram_tensor("out", [TOKS, HID], dt_in, kind="ExternalOutput")

    with ExitStack() as ctx:
        mega = ctx.enter_context(nc.sbuf_tensor([P, PACK_COLS], dt_in))
        out_sb = ctx.enter_context(nc.sbuf_tensor([P, NROW * HID], dt_in))
        vT = ctx.enter_context(nc.sbuf_tensor([D, TOKS], dt_in))
        warm = ctx.enter_context(nc.sbuf_tensor([P, 2], dt_in))
        psv0 = ctx.enter_context(nc.psum_tensor([D, TGK]))
        psv1 = ctx.enter_context(nc.psum_tensor([D, TGK]))
        psB = ctx.enter_context(nc.psum_tensor([P, NB * CD]))
        # one semaphore per input piece: a shared counting semaphore races
        # (the 16 per-engine completion increments of piece k+1 are
        # indistinguishable from piece k's), so "s >= 16*(k+1)" does not
        # guarantee piece k fully landed under engine skew.
        s_piece = [
            ctx.enter_context(nc.semaphore(name=f"s_piece{i}"))
            for i in range(len(pieces))
        ]
        s_pe = ctx.enter_context(nc.semaphore())
        s_vt = ctx.enter_context(nc.semaphore())
        s_cd = ctx.enter_context(nc.semaphore())
        s_ca = ctx.enter_context(nc.semaphore())
        s_store = ctx.enter_context(nc.semaphore())
        block = ctx.enter_context(nc.Block())

        psv = [psv0, psv1]
        sem_of = {"V": s_cd, "A": s_ca}

        def wv_chunk(c):
            return mega[:, c * D:(c + 1) * D]

        def wos_ap(ct):
            return mega[:D, WV_COLS + ct * CD:WV_COLS + (ct + 1) * CD]

        def ht_ap(g, c):
            lo = HT0 + g * KC * TGK + c * TGK
            return mega[:, lo:lo + TGK]

        def pair_aps(p):
            """dst/src APs for the 2-tile cast of pair p (tiles 2p, 2p+1)."""
            i0 = 2 * p
            g, k0 = divmod(i0, 2 * NCT)
            rb_l, ct0 = divmod(k0, NCT)
            slot0 = i0 % NB          # even i0 -> slot0 in {0,2,4}: contiguous
            rb = g * 2 + rb_l
            dst = out_sb[:, rb * HID + ct0 * CD:rb * HID + (ct0 + 2) * CD]
            src = psB[:, slot0 * CD:(slot0 + 2) * CD]
            return dst, src

        @block.sync
        def _(sync):
            for idx, (key, (lo, hi)) in enumerate(pieces):
                sync.dma_start(out=mega[:, lo:hi], in_=pack[:, lo:hi]).then_inc(
                    s_piece[idx], 16
                )

        @block.tensor
        def _(tensor):
            for kind, g in ORDER:
                if kind == "A":
                    # stage A: accumulate vT(g) over 32 k-chunks as pieces land
                    if g >= 2:
                        tensor.wait_ge(s_vt, g - 1)  # WAR: psv[g%2] cast done
                    for c in range(KC):
                        if c % CPL == 0:
                            if g == 0 and c == 0:
                                tensor.wait_ge(s_piece[piece_idx["wv"]], 16)
                            tensor.wait_ge(
                                s_piece[piece_idx[("ht", g, c // CPL)]], 16
                            )
                        mm = tensor.matmul(
                            psv[g % 2][:, :], wv_chunk(c), ht_ap(g, c),
                            start=(c == 0), stop=(c == KC - 1),
                        )
                        if c == KC - 1:
                            mm.then_inc(s_pe, 1)
                else:
                    # stage B: 2 row-blocks x 8 column tiles
                    tensor.wait_ge(s_vt, g + 1)  # vT(g) cast landed in SBUF
                    for k in range(2 * NCT):
                        rb_l, ct = divmod(k, NCT)
                        i = g * 2 * NCT + k
                        if g == 0 and rb_l == 0 and ct % 2 == 0:
                            tensor.wait_ge(
                                s_piece[piece_idx[("wos", ct // 2)]], 16
                            )
                        if i >= NB and i % 2 == 0:
                            # WAR: the pair cast holding slots (i%NB, i%NB+1)
                            # (tiles i-NB, i-NB+1) must have drained
                            q = (i - NB) // 2
                            tensor.wait_ge(sem_of[eng_of_pair[q]],
                                           pcum[eng_of_pair[q]][q])
                        slot = i % NB
                        rb = g * 2 + rb_l
                        tensor.matmul(
                            psB[:, slot * CD:(slot + 1) * CD],
                            vT[:, rb * P:(rb + 1) * P],
                            wos_ap(ct),
                            start=True, stop=True,
                        ).then_inc(s_pe, 1)

        @block.vector
        def _(vector):
            for p in range(n_pairs):
                if eng_of_pair[p] != "V":
                    continue
                g, k1 = divmod(2 * p + 1, 2 * NCT)
                vector.wait_ge(s_pe, tick_b[(g, k1)])
                dst, src = pair_aps(p)
                vector.tensor_copy(out=dst, in_=src).then_inc(s_cd, 1)

        @block.scalar
        def _(scalar):
            # dummy copy: loads the activation table during kernel startup so
            # the first real cast doesn't eat the ~1.5us ACT_TABLE_LOAD
            scalar.copy(out=warm[:, 0:1], in_=warm[:, 1:2])
            for kind, g in ORDER:
                if kind == "A":
                    scalar.wait_ge(s_pe, tick_a[g])
                    scalar.copy(
                        out=vT[:, g * TGK:(g + 1) * TGK], in_=psv[g % 2][:, :]
                    ).then_inc(s_vt, 1)
                else:
                    for p in range(g * NCT, (g + 1) * NCT):
                        if eng_of_pair[p] != "A":
                            continue
                        k1 = (2 * p + 1) % (2 * NCT)
                        scalar.wait_ge(s_pe, tick_b[(g, k1)])
                        dst, src = pair_aps(p)
                        scalar.copy(out=dst, in_=src).then_inc(s_ca, 1)

        @block.gpsimd
        def _(gpsimd):
            for rb in range(NROW - 1):
                gpsimd.wait_ge(s_cd, 2 * (rb + 1))
                gpsimd.wait_ge(s_ca, 2 * (rb + 1))
                gpsimd.dma_start(
                    out=out[rb * P:(rb + 1) * P, :],
                    in_=out_sb[:, rb * HID:(rb + 1) * HID],
                ).then_inc(s_store, 16)
            # last row-block in two halves for a shorter kernel tail
            rb = NROW - 1
            for h in range(2):
                gpsimd.wait_ge(s_cd, 15 + h)
                gpsimd.wait_ge(s_ca, 15 + h)
                gpsimd.dma_start(
                    out=out[rb * P:(rb + 1) * P, h * (HID // 2):(h + 1) * (HID // 2)],
                    in_=out_sb[:, rb * HID + h * (HID // 2):rb * HID + (h + 1) * (HID // 2)],
                ).then_inc(s_store, 16)
            gpsimd.wait_ge(s_store, 16 * (NROW + 1))
    return nc


def kernel(hidden_states, cos, sin, Wq, Wk, Wv, Wo):
    global LAST_RESULT
    import ml_dtypes
    np_bf16 = ml_dtypes.bfloat16

    if "nc" not in _CACHE:
        _CACHE["nc"] = _build()
    nc = _CACHE["nc"]

    hidden_states = np.asarray(hidden_states, dtype=np.float32)
    Wv = np.asarray(Wv, dtype=np.float32)
    Wo = np.asarray(Wo, dtype=np.float32)

    flat = hidden_states.reshape(B * T, HID)
    # Wv^T chunks: pack[p, c*64+d] = Wv[d, c*128+p]
    wv_part = np.ascontiguousarray(
        Wv.reshape(D, KC, P).transpose(2, 1, 0).reshape(P, KC * D)
    ).astype(np_bf16)
    # Wo_sum^T on partitions 0..63: pack[d, j] = sum_h Wo[j, 64h+d]
    woS = Wo.reshape(HID, HID // D, D).sum(axis=1, dtype=np.float32).T  # [64, 4096]
    woS_part = np.zeros((P, WOS_COLS), dtype=np_bf16)
    woS_part[:D, :] = woS.astype(np_bf16)

    in_maps = []
    for j in range(N_CORES):
        blk = flat[j * TOKS:(j + 1) * TOKS, :]          # [1024, 4096]
        # ht group-major: pack[p, g*KC*TGK + c*TGK + t'] = blk[g*TGK+t', c*128+p]
        ht_part = np.ascontiguousarray(
            blk.reshape(G, TGK, KC, P).transpose(3, 0, 2, 1).reshape(P, HT_COLS)
        ).astype(np_bf16)
        packed = np.concatenate([wv_part, woS_part, ht_part], axis=1)
        in_maps.append({"pack": np.ascontiguousarray(packed)})

    LAST_RESULT = run_bass_kernel_spmd(nc, in_maps, core_ids=list(range(N_CORES)))
    outs = [np.asarray(LAST_RESULT.results[j]["out"]).astype(np.float32)
            for j in range(N_CORES)]
    return np.concatenate(outs, axis=0).reshape(B, T, HID)
